# revision 12
# baseline (speedup 1.0000x reference)
"""CLSTMCell fused cell kernel for 8 Trainium2 NeuronCores.

Data-parallel over the batch: each of the 8 cores processes a 512-row batch
shard; the four (D,4U) kernels and biases are replicated to every core.

Math (per batch shard, D = U = 1024):
    zr = xr@R + xi@I + hr@Rr + hi@Ir + br          [512, 4096]
    zi = xi@R - xr@I + hi@Rr - hr@Ir + bi          [512, 4096]
    per gate g (i,f,c,o) and half (r from zr, i from zi):
        i,f,o -> hard_sigmoid(z) = clip(0.2 z + 0.5, 0, 1);  c~ -> tanh(z)
    c = f*c_tm1 + i*tanh(c~);  h = o*tanh(c)

Device layout: output columns (n) on SBUF partitions, batch on the free dim.
Each matmul takes a [128k, 128n] weight tile as the stationary operand and a
transposed-activation block [128k, 512b] as the moving operand at float32r
(full-rate fp32), accumulating zT[n0:n0+128, :] over the 32 k-blocks. The
zr/zi pair shares each loaded weight tile, amortizing LDWEIGHTS. One phase =
one 128-wide u-block: all four gate psums for both halves live in the 8 PSUM
banks, so the gate combine needs no cross-phase staging, and biases are
per-partition scalars. Host-side work is layout only (slice/transpose/
reshape); zi's -xr/-hr blocks are negated once on device.
"""

import sys

sys.path.insert(0, "/opt/trn_rl_repo")

import numpy as np

import concourse.bacc as bacc
import concourse.mybir as mybir
import concourse.tile as tile
from concourse.bass_utils import run_bass_kernel_spmd

N_CORES = 8
B, D, U = 4096, 1024, 1024
BS = B // N_CORES          # batch rows per core
P = 128                    # SBUF partitions
KB = (2 * D + 2 * U) // P  # 32 contraction blocks of 128
NJ = U // P                # 8 u-blocks (phases)
F32 = mybir.dt.float32
F32R = mybir.dt.float32r
ADD = mybir.AluOpType.add
MULT = mybir.AluOpType.mult
MIN = mybir.AluOpType.min
MAX = mybir.AluOpType.max
TANH = None  # set lazily (mybir.ActivationFunctionType.Tanh)

_CACHE = {}


def _build():
    nc = bacc.Bacc("TRN2", target_bir_lowering=False, debug=False,
                   num_devices=N_CORES)
    Tanh = mybir.ActivationFunctionType.Tanh

    din = {}
    for name in ("xrT", "xiT", "hrT", "hiT"):
        din[name] = nc.dram_tensor(name, [D, BS], F32R, kind="ExternalInput").ap()
    din["c_prevT"] = nc.dram_tensor("c_prevT", [2 * U, BS], F32,
                                    kind="ExternalInput").ap()
    din["wperm"] = nc.dram_tensor("wperm", [NJ * KB * P, 4 * P], F32R,
                                  kind="ExternalInput").ap()
    din["brT"] = nc.dram_tensor("brT", [P, KB], F32, kind="ExternalInput").ap()
    din["biT"] = nc.dram_tensor("biT", [P, KB], F32, kind="ExternalInput").ap()
    h_outT = nc.dram_tensor("h_outT", [2 * U, BS], F32, kind="ExternalOutput").ap()
    c_outT = nc.dram_tensor("c_outT", [2 * U, BS], F32, kind="ExternalOutput").ap()

    with tile.TileContext(nc) as tc:
        with (
            tc.tile_pool(name="acts", bufs=48) as acts,
            tc.tile_pool(name="bias", bufs=4) as bias_p,
            tc.tile_pool(name="wpool", bufs=16) as wpool,
            tc.tile_pool(name="cprev", bufs=4) as cpool,
            tc.tile_pool(name="gat", bufs=6) as gat_p,
            tc.tile_pool(name="tmp", bufs=6) as tmp_p,
            tc.tile_pool(name="outs", bufs=8) as out_p,
            tc.tile_pool(name="psum", bufs=8, space="PSUM") as psum_p,
        ):
            # --- resident transposed-activation blocks, loaded lazily -------
            act_tiles = {}   # (src_name, block) -> tile
            neg_tiles = {}

            def act(name, j):
                t = act_tiles.get((name, j))
                if t is None:
                    t = acts.tile([P, BS], F32R, tag="acts", name=f"{name}{j}")
                    nc.sync.dma_start(t[:], din[name][j * P:(j + 1) * P, :])
                    act_tiles[(name, j)] = t
                return t

            def nact(name, j):
                t = neg_tiles.get((name, j))
                if t is None:
                    t = acts.tile([P, BS], F32R, tag="acts", name=f"n{name}{j}")
                    nc.vector.tensor_scalar_mul(t[:], act(name, j)[:], -1.0)
                    neg_tiles[(name, j)] = t
                return t

            A_SRC = ("xrT", "xiT", "hrT", "hiT")   # zr moving blocks by k//8
            B_SRC = ("xiT", "xrT", "hiT", "hrT")   # zi moving blocks (neg on 1,3)

            def a_block(k):
                return act(A_SRC[k // 8], k % 8)

            def b_block(k):
                name = B_SRC[k // 8]
                if (k // 8) % 2 == 1:
                    return nact(name, k % 8)
                return act(name, k % 8)

            # --- per-partition bias tiles [128, 32]; col m = n-block index --
            # raw for the c~ gate; 0.2*b + 0.5 pre-folded for the hsig gates.
            # Emitted lazily (first combine) so startup DMAs aren't queued
            # behind them.
            braw, bhs = [], []

            def emit_bias():
                for name in ("brT", "biT"):
                    t = bias_p.tile([P, KB], F32, tag="bias",
                                    name=f"braw_{name}")
                    nc.sync.dma_start(t[:], din[name][:, :])
                    braw.append(t)
                    t2 = bias_p.tile([P, KB], F32, tag="bias",
                                     name=f"bhs_{name}")
                    nc.vector.tensor_scalar(t2[:], t[:], 0.2, 0.5, MULT, ADD)
                    bhs.append(t2)

            # --- main loop: one phase per 128-wide u-block ------------------
            for j in range(NJ):
                # psum groups: (gate, z) -> zT[g*U + j*128 : .. , :] (8 banks)
                ps = {(g, z): psum_p.tile([P, BS], F32, tag="ps",
                                          name=f"ps_{j}_{g}_{z}")
                      for g in range(4) for z in range(2)}
                # --- gate combine, per half (emitted via combine()) -------
                cps = {}

                def emit_cps(j=j):
                    for z in range(2):
                        rows0 = z * U + j * P
                        cp = cpool.tile([P, BS], F32, tag="cprev",
                                        name=f"cp_{j}_{z}")
                        nc.sync.dma_start(
                            cp[:], din["c_prevT"][rows0:rows0 + P, :])
                        cps[z] = cp

                def combine(z, j=j, ps=ps):
                    if not braw:
                        emit_bias()
                    rows0 = z * U + j * P
                    cp = cps[z]

                    def hsgate(g):
                        t = gat_p.tile([P, BS], F32, tag="gat",
                                       name=f"hs_{j}_{z}_{g}")
                        bia = bhs[z][:, g * NJ + j:g * NJ + j + 1]
                        nc.vector.tensor_scalar(t[:], ps[(g, z)][:],
                                                0.2, bia, MULT, ADD)
                        nc.vector.tensor_scalar(t[:], t[:], 1.0, 0.0, MIN, MAX)
                        return t

                    # c~ = tanh(z_c + b_c); o computed early so only the
                    # final h-mul trails tanh(c) on the DVE
                    tcin = tmp_p.tile([P, BS], F32, tag="tmp",
                                      name=f"tcin_{j}_{z}")
                    nc.vector.tensor_scalar(
                        tcin[:], ps[(2, z)][:], 1.0,
                        braw[z][:, 2 * NJ + j:2 * NJ + j + 1], MULT, ADD)
                    tct = tmp_p.tile([P, BS], F32, tag="tmp",
                                     name=f"tct_{j}_{z}")
                    nc.scalar.activation(tct[:], tcin[:], Tanh)

                    o_t = hsgate(3)
                    f_t = hsgate(1)
                    i_t = hsgate(0)
                    # c = f*c_prev + i*tanh(c~); products go to GpSimd in the
                    # kernel tail so the DVE chain after the last matmul is
                    # shorter
                    last = (j == NJ - 1 and z == 1)
                    eng = nc.gpsimd if last else nc.vector
                    t1 = tmp_p.tile([P, BS], F32, tag="tmp", name=f"t1_{j}_{z}")
                    eng.tensor_tensor(t1[:], f_t[:], cp[:], MULT)
                    t2 = tmp_p.tile([P, BS], F32, tag="tmp", name=f"t2_{j}_{z}")
                    eng.tensor_tensor(t2[:], i_t[:], tct[:], MULT)
                    cn = out_p.tile([P, BS], F32, tag="out", name=f"cn_{j}_{z}")
                    nc.vector.tensor_tensor(cn[:], t1[:], t2[:], ADD)
                    nc.sync.dma_start(c_outT[rows0:rows0 + P, :], cn[:])
                    # h = o * tanh(c)
                    tc2 = tmp_p.tile([P, BS], F32, tag="tmp", name=f"tc2_{j}_{z}")
                    nc.scalar.activation(tc2[:], cn[:], Tanh)
                    hn = out_p.tile([P, BS], F32, tag="out", name=f"hn_{j}_{z}")
                    nc.vector.tensor_tensor(hn[:], o_t[:], tc2[:], MULT)
                    nc.sync.dma_start(h_outT[rows0:rows0 + P, :], hn[:])

                # z=0 leads z=1 by LAG k-steps so the real half's gate
                # combine overlaps the imaginary half's trailing matmuls
                LAG = 4
                wts = {}
                for t in range(KB + LAG):
                    if t < KB:
                        k = t
                        if j == 0:
                            a_block(k)  # first-use DMA just ahead of its step
                        wt = wpool.tile([P, 4 * P], F32R, tag="w",
                                        name=f"w_{j}_{k}")
                        row0 = (j * KB + k) * P
                        nc.sync.dma_start(wt[:],
                                          din["wperm"][row0:row0 + P, :])
                        wts[k] = wt
                        am = a_block(k)[:]
                        for g in range(4):
                            nc.tensor.matmul(ps[(g, 0)][:],
                                             wt[:, g * P:(g + 1) * P], am,
                                             start=(k == 0), stop=(k == KB - 1))
                    if t >= LAG:
                        k = t - LAG
                        if j == 0:
                            b_block(k)
                        bm = b_block(k)[:]
                        wt = wts.pop(k)
                        for g in range(4):
                            nc.tensor.matmul(ps[(g, 1)][:],
                                             wt[:, g * P:(g + 1) * P], bm,
                                             start=(k == 0), stop=(k == KB - 1))
                    if t == 2:
                        emit_cps()
                    if t == KB - 1:
                        combine(0)
                combine(1)

    nc.compile()
    return nc


def _in_maps(inputs, h_tm1, c_tm1, wr, wi, wrr, wir, br, bi):
    brT = np.ascontiguousarray(br.reshape(KB, P).T)
    biT = np.ascontiguousarray(bi.reshape(KB, P).T)
    # wperm[j, k, p, g*128+c] = W_src(k)[(k%8)*128+p, g*1024+j*128+c]
    wall = np.stack([wr, wi, wrr, wir])        # [s, 1024, 4096]
    v = wall.reshape(4, 8, P, 4, NJ, P)        # [s, kr, p, g, j, c]
    wperm = np.ascontiguousarray(
        v.transpose(4, 0, 1, 2, 3, 5).reshape(NJ * KB * P, 4 * P))
    maps = []
    for c in range(N_CORES):
        rows = slice(c * BS, (c + 1) * BS)
        maps.append({
            "xrT": np.ascontiguousarray(inputs[rows, :D].T),
            "xiT": np.ascontiguousarray(inputs[rows, D:].T),
            "hrT": np.ascontiguousarray(h_tm1[rows, :U].T),
            "hiT": np.ascontiguousarray(h_tm1[rows, U:].T),
            "c_prevT": np.ascontiguousarray(c_tm1[rows].T),
            "wperm": wperm,
            "brT": brT, "biT": biT,
        })
    return maps


def kernel(inputs, h_tm1, c_tm1, real_kernel, imaginary_kernel,
           real_recurrent_kernel, imaginary_recurrent_kernel,
           real_bias, imaginary_bias):
    if "nc" not in _CACHE:
        _CACHE["nc"] = _build()
    nc = _CACHE["nc"]

    maps = _in_maps(
        np.ascontiguousarray(inputs, dtype=np.float32),
        np.ascontiguousarray(h_tm1, dtype=np.float32),
        np.ascontiguousarray(c_tm1, dtype=np.float32),
        np.ascontiguousarray(real_kernel, dtype=np.float32),
        np.ascontiguousarray(imaginary_kernel, dtype=np.float32),
        np.ascontiguousarray(real_recurrent_kernel, dtype=np.float32),
        np.ascontiguousarray(imaginary_recurrent_kernel, dtype=np.float32),
        np.ascontiguousarray(real_bias, dtype=np.float32),
        np.ascontiguousarray(imaginary_bias, dtype=np.float32),
    )
    res = run_bass_kernel_spmd(nc, maps, list(range(N_CORES)))
    h = np.concatenate(
        [res.results[c]["h_outT"].T for c in range(N_CORES)], axis=0)
    c = np.concatenate(
        [res.results[c]["c_outT"].T for c in range(N_CORES)], axis=0)
    return np.ascontiguousarray(h), np.ascontiguousarray(c)


# revision 14
# speedup vs baseline: 1.0049x; 1.0049x over previous
"""CLSTMCell fused cell kernel for 8 Trainium2 NeuronCores.

Data-parallel over the batch: each of the 8 cores processes a 512-row batch
shard; the four (D,4U) kernels and biases are replicated to every core.

Math (per batch shard, D = U = 1024):
    zr = xr@R + xi@I + hr@Rr + hi@Ir + br          [512, 4096]
    zi = xi@R - xr@I + hi@Rr - hr@Ir + bi          [512, 4096]
    per gate g (i,f,c,o) and half (r from zr, i from zi):
        i,f,o -> hard_sigmoid(z) = clip(0.2 z + 0.5, 0, 1);  c~ -> tanh(z)
    c = f*c_tm1 + i*tanh(c~);  h = o*tanh(c)

Device layout: output columns (n) on SBUF partitions, batch on the free dim.
Each matmul takes a [128k, 128n] weight tile as the stationary operand and a
transposed-activation block [128k, 512b] as the moving operand at float32r
(full-rate fp32), accumulating zT[n0:n0+128, :] over the 32 k-blocks. The
zr/zi pair shares each loaded weight tile, amortizing LDWEIGHTS. One phase =
one 128-wide u-block: all four gate psums for both halves live in the 8 PSUM
banks, so the gate combine needs no cross-phase staging, and biases are
per-partition scalars. Host-side work is layout only (slice/transpose/
reshape); zi's -xr/-hr blocks are negated once on device.
"""

import sys

sys.path.insert(0, "/opt/trn_rl_repo")

import numpy as np

import concourse.bacc as bacc
import concourse.mybir as mybir
import concourse.tile as tile
from concourse.bass_utils import run_bass_kernel_spmd

N_CORES = 8
B, D, U = 4096, 1024, 1024
BS = B // N_CORES          # batch rows per core
P = 128                    # SBUF partitions
KB = (2 * D + 2 * U) // P  # 32 contraction blocks of 128
NJ = U // P                # 8 u-blocks (phases)
F32 = mybir.dt.float32
F32R = mybir.dt.float32r
ADD = mybir.AluOpType.add
MULT = mybir.AluOpType.mult
MIN = mybir.AluOpType.min
MAX = mybir.AluOpType.max
TANH = None  # set lazily (mybir.ActivationFunctionType.Tanh)

_CACHE = {}


def _build():
    nc = bacc.Bacc("TRN2", target_bir_lowering=False, debug=False,
                   num_devices=N_CORES)
    Tanh = mybir.ActivationFunctionType.Tanh

    din = {}
    for name in ("xrT", "xiT", "hrT", "hiT"):
        din[name] = nc.dram_tensor(name, [D, BS], F32R, kind="ExternalInput").ap()
    din["c_prevT"] = nc.dram_tensor("c_prevT", [2 * U, BS], F32,
                                    kind="ExternalInput").ap()
    din["wperm"] = nc.dram_tensor("wperm", [NJ * KB * P, 4 * P], F32R,
                                  kind="ExternalInput").ap()
    din["brT"] = nc.dram_tensor("brT", [P, KB], F32, kind="ExternalInput").ap()
    din["biT"] = nc.dram_tensor("biT", [P, KB], F32, kind="ExternalInput").ap()
    h_outT = nc.dram_tensor("h_outT", [2 * U, BS], F32, kind="ExternalOutput").ap()
    c_outT = nc.dram_tensor("c_outT", [2 * U, BS], F32, kind="ExternalOutput").ap()

    with tile.TileContext(nc) as tc:
        with (
            tc.tile_pool(name="acts", bufs=48) as acts,
            tc.tile_pool(name="bias", bufs=4) as bias_p,
            tc.tile_pool(name="wpool", bufs=16) as wpool,
            tc.tile_pool(name="cprev", bufs=4) as cpool,
            tc.tile_pool(name="gat", bufs=6) as gat_p,
            tc.tile_pool(name="tmp", bufs=6) as tmp_p,
            tc.tile_pool(name="outs", bufs=8) as out_p,
            tc.tile_pool(name="psum", bufs=8, space="PSUM") as psum_p,
        ):
            # --- resident transposed-activation blocks, loaded lazily -------
            act_tiles = {}   # (src_name, block) -> tile
            neg_tiles = {}

            def act(name, j):
                t = act_tiles.get((name, j))
                if t is None:
                    t = acts.tile([P, BS], F32R, tag="acts", name=f"{name}{j}")
                    nc.sync.dma_start(t[:], din[name][j * P:(j + 1) * P, :])
                    act_tiles[(name, j)] = t
                return t

            def nact(name, j):
                t = neg_tiles.get((name, j))
                if t is None:
                    t = acts.tile([P, BS], F32R, tag="acts", name=f"n{name}{j}")
                    nc.vector.tensor_scalar_mul(t[:], act(name, j)[:], -1.0)
                    neg_tiles[(name, j)] = t
                return t

            A_SRC = ("xrT", "xiT", "hrT", "hiT")   # zr moving blocks by k//8
            B_SRC = ("xiT", "xrT", "hiT", "hrT")   # zi moving blocks (neg on 1,3)

            def a_block(k):
                return act(A_SRC[k // 8], k % 8)

            def b_block(k):
                name = B_SRC[k // 8]
                if (k // 8) % 2 == 1:
                    return nact(name, k % 8)
                return act(name, k % 8)

            # --- per-partition bias tiles [128, 32]; col m = n-block index --
            # raw for the c~ gate; 0.2*b + 0.5 pre-folded for the hsig gates.
            # Emitted lazily (first combine) so startup DMAs aren't queued
            # behind them.
            braw, bhs = [], []

            def emit_bias():
                for name in ("brT", "biT"):
                    t = bias_p.tile([P, KB], F32, tag="bias",
                                    name=f"braw_{name}")
                    nc.sync.dma_start(t[:], din[name][:, :])
                    braw.append(t)
                    t2 = bias_p.tile([P, KB], F32, tag="bias",
                                     name=f"bhs_{name}")
                    nc.vector.tensor_scalar(t2[:], t[:], 0.2, 0.5, MULT, ADD)
                    bhs.append(t2)

            # --- main loop: one phase per 128-wide u-block ------------------
            for j in range(NJ):
                # psum groups: (gate, z) -> zT[g*U + j*128 : .. , :] (8 banks)
                ps = {(g, z): psum_p.tile([P, BS], F32, tag="ps",
                                          name=f"ps_{j}_{g}_{z}")
                      for g in range(4) for z in range(2)}
                # --- gate combine, per half (emitted via combine()) -------
                cps = {}

                def emit_cps(j=j):
                    for z in range(2):
                        rows0 = z * U + j * P
                        cp = cpool.tile([P, BS], F32, tag="cprev",
                                        name=f"cp_{j}_{z}")
                        nc.sync.dma_start(
                            cp[:], din["c_prevT"][rows0:rows0 + P, :])
                        cps[z] = cp

                tc2s = {}

                def combine_ci(z, j=j, ps=ps):
                    if not braw:
                        emit_bias()
                    rows0 = z * U + j * P
                    cp = cps[z]

                    def hsgate(g):
                        t = gat_p.tile([P, BS], F32, tag="gat",
                                       name=f"hs_{j}_{z}_{g}")
                        bia = bhs[z][:, g * NJ + j:g * NJ + j + 1]
                        nc.vector.tensor_scalar(t[:], ps[(g, z)][:],
                                                0.2, bia, MULT, ADD)
                        nc.vector.tensor_scalar(t[:], t[:], 1.0, 0.0, MIN, MAX)
                        return t

                    # c~ = tanh(z_c + b_c); o computed early so only the
                    # final h-mul trails tanh(c) on the DVE
                    tcin = tmp_p.tile([P, BS], F32, tag="tmp",
                                      name=f"tcin_{j}_{z}")
                    nc.vector.tensor_scalar(
                        tcin[:], ps[(2, z)][:], 1.0,
                        braw[z][:, 2 * NJ + j:2 * NJ + j + 1], MULT, ADD)
                    tct = tmp_p.tile([P, BS], F32, tag="tmp",
                                     name=f"tct_{j}_{z}")
                    nc.scalar.activation(tct[:], tcin[:], Tanh)

                    f_t = hsgate(1)
                    i_t = hsgate(0)
                    # c = f*c_prev + i*tanh(c~); products go to GpSimd in the
                    # kernel tail so the DVE chain after the last matmul is
                    # shorter
                    last = (j == NJ - 1 and z == 1)
                    eng = nc.gpsimd if last else nc.vector
                    t1 = tmp_p.tile([P, BS], F32, tag="tmp", name=f"t1_{j}_{z}")
                    eng.tensor_tensor(t1[:], f_t[:], cp[:], MULT)
                    t2 = tmp_p.tile([P, BS], F32, tag="tmp", name=f"t2_{j}_{z}")
                    eng.tensor_tensor(t2[:], i_t[:], tct[:], MULT)
                    cn = out_p.tile([P, BS], F32, tag="out", name=f"cn_{j}_{z}")
                    nc.vector.tensor_tensor(cn[:], t1[:], t2[:], ADD)
                    nc.sync.dma_start(c_outT[rows0:rows0 + P, :], cn[:])
                    tc2 = tmp_p.tile([P, BS], F32, tag="tmp", name=f"tc2_{j}_{z}")
                    nc.scalar.activation(tc2[:], cn[:], Tanh)
                    tc2s[z] = (tc2, hsgate)

                def combine_o(z, j=j, ps=ps):
                    rows0 = z * U + j * P
                    tc2, hsgate = tc2s[z]
                    o_t = hsgate(3)
                    hn = out_p.tile([P, BS], F32, tag="out", name=f"hn_{j}_{z}")
                    nc.vector.tensor_tensor(hn[:], o_t[:], tc2[:], MULT)
                    nc.sync.dma_start(h_outT[rows0:rows0 + P, :], hn[:])

                # staggered k-sweeps per group class: f/c~/i run at lag
                # 0 (real) / 4 (imag); the o-gate groups trail at lag 8 / 12,
                # so after the very last matmul only the short o->h chain
                # remains, and each half's c-chain hides under later matmuls
                LAG, OLAG = 4, 8
                wts = {}
                for t in range(KB + OLAG + LAG):
                    if t < KB:
                        k = t
                        if j == 0:
                            a_block(k)  # first-use DMA just ahead of its step
                        wt = wpool.tile([P, 4 * P], F32R, tag="w",
                                        name=f"w_{j}_{k}")
                        row0 = (j * KB + k) * P
                        nc.scalar.dma_start(wt[:],
                                            din["wperm"][row0:row0 + P, :])
                        wts[k] = wt
                        am = a_block(k)[:]
                        for g in (0, 1, 2):
                            nc.tensor.matmul(ps[(g, 0)][:],
                                             wt[:, g * P:(g + 1) * P], am,
                                             start=(k == 0), stop=(k == KB - 1))
                    if LAG <= t < KB + LAG:
                        k = t - LAG
                        if j == 0:
                            b_block(k)
                        bm = b_block(k)[:]
                        wt = wts[k]
                        for g in (0, 1, 2):
                            nc.tensor.matmul(ps[(g, 1)][:],
                                             wt[:, g * P:(g + 1) * P], bm,
                                             start=(k == 0), stop=(k == KB - 1))
                    if OLAG <= t < KB + OLAG:
                        k = t - OLAG
                        wt = wts[k]
                        nc.tensor.matmul(ps[(3, 0)][:],
                                         wt[:, 3 * P:4 * P], a_block(k)[:],
                                         start=(k == 0), stop=(k == KB - 1))
                    if OLAG + LAG <= t < KB + OLAG + LAG:
                        k = t - OLAG - LAG
                        wt = wts.pop(k)
                        nc.tensor.matmul(ps[(3, 1)][:],
                                         wt[:, 3 * P:4 * P], b_block(k)[:],
                                         start=(k == 0), stop=(k == KB - 1))
                    if t == 2:
                        emit_cps()
                    if t == KB - 1:
                        combine_ci(0)
                    if t == KB + LAG - 1:
                        combine_ci(1)
                    if t == KB + OLAG - 1:
                        combine_o(0)
                combine_o(1)

    nc.compile()
    return nc


def _in_maps(inputs, h_tm1, c_tm1, wr, wi, wrr, wir, br, bi):
    brT = np.ascontiguousarray(br.reshape(KB, P).T)
    biT = np.ascontiguousarray(bi.reshape(KB, P).T)
    # wperm[j, k, p, g*128+c] = W_src(k)[(k%8)*128+p, g*1024+j*128+c]
    wall = np.stack([wr, wi, wrr, wir])        # [s, 1024, 4096]
    v = wall.reshape(4, 8, P, 4, NJ, P)        # [s, kr, p, g, j, c]
    wperm = np.ascontiguousarray(
        v.transpose(4, 0, 1, 2, 3, 5).reshape(NJ * KB * P, 4 * P))
    maps = []
    for c in range(N_CORES):
        rows = slice(c * BS, (c + 1) * BS)
        maps.append({
            "xrT": np.ascontiguousarray(inputs[rows, :D].T),
            "xiT": np.ascontiguousarray(inputs[rows, D:].T),
            "hrT": np.ascontiguousarray(h_tm1[rows, :U].T),
            "hiT": np.ascontiguousarray(h_tm1[rows, U:].T),
            "c_prevT": np.ascontiguousarray(c_tm1[rows].T),
            "wperm": wperm,
            "brT": brT, "biT": biT,
        })
    return maps


def kernel(inputs, h_tm1, c_tm1, real_kernel, imaginary_kernel,
           real_recurrent_kernel, imaginary_recurrent_kernel,
           real_bias, imaginary_bias):
    if "nc" not in _CACHE:
        _CACHE["nc"] = _build()
    nc = _CACHE["nc"]

    maps = _in_maps(
        np.ascontiguousarray(inputs, dtype=np.float32),
        np.ascontiguousarray(h_tm1, dtype=np.float32),
        np.ascontiguousarray(c_tm1, dtype=np.float32),
        np.ascontiguousarray(real_kernel, dtype=np.float32),
        np.ascontiguousarray(imaginary_kernel, dtype=np.float32),
        np.ascontiguousarray(real_recurrent_kernel, dtype=np.float32),
        np.ascontiguousarray(imaginary_recurrent_kernel, dtype=np.float32),
        np.ascontiguousarray(real_bias, dtype=np.float32),
        np.ascontiguousarray(imaginary_bias, dtype=np.float32),
    )
    res = run_bass_kernel_spmd(nc, maps, list(range(N_CORES)))
    h = np.concatenate(
        [res.results[c]["h_outT"].T for c in range(N_CORES)], axis=0)
    c = np.concatenate(
        [res.results[c]["c_outT"].T for c in range(N_CORES)], axis=0)
    return np.ascontiguousarray(h), np.ascontiguousarray(c)
